# revision 40
# baseline (speedup 1.0000x reference)
"""Gated linear attention (GLA) Bass kernel for Trainium2, 8 NeuronCores.

Sharding: one core per (batch, head) pair -- B=2 x H=4 = 8 cores.
Each core computes its head's full pipeline with a chunked-parallel form of
the gated recurrence (chunk = 128), entirely on-device.

v4 design (v0 baseline 147us, v2 74us, v3 82us):
  - all matmuls bf16 (1 cycle/row on PE), fp32 PSUM accumulation; host
    ships x^T and weights pre-cast to bf16.
  - gate chain packed two 512-token slices per instruction (the gate is
    only dk=64 wide, so slices j and j+1 share one 128-partition tile):
    halves the ACT/DVE work and the cumsum-scan count.
  - inter-chunk state recurrence S_c = (S_{c-1} + D_c) * e_c runs as one
    scalar_tensor_tensor per chunk (S_{c-1}*e_c + D'_c with e_c folded
    into k-tilde before the D matmul), with D'_c read straight from
    PSUM -- no state eviction pass, no scan barrier between phases.
  - O computed transposed ([dv, t]): no per-chunk PE transpose for the
    output head; RMSNorm sum-of-squares via ones-matmul; one global rstd,
    one gated eviction, one output DMA.
  - weights and x stream on separate DMA queues so the first gate matmul
    isn't stuck behind the full weight load.
"""
import sys, os
sys.path.insert(0, "/opt/trn_rl_repo")

import numpy as np

B, T, D = 2, 2048, 512
H = 4
dk, dv = 64, 128          # per-head key/value dims
C = 128                   # chunk length
GATE_NORM = 16.0
EPS = 1e-5
SCALE = dk ** -0.5

_CACHE = {}


def build(t=T):
    import concourse.bass as bass  # noqa: F401
    from concourse import bacc, mybir
    import concourse.tile as tile
    import concourse.hw_specs as hw_specs

    F32 = mybir.dt.float32
    BF16 = mybir.dt.bfloat16
    AF = mybir.ActivationFunctionType
    OP = mybir.AluOpType

    # Steer the activation-table chooser so every func we use (Exp, Ln,
    # Square, Copy, Identity) resolves to natural_log_exp_and_others --
    # otherwise Exp->exp_and_others vs Ln->natural_log thrashes
    # ACT_TABLE_LOADs (~1.3us each) between every pair.
    need = {AF.Exp, AF.Ln, AF.Square, AF.Copy, AF.Identity}
    keep = "natural_log_exp_and_others"
    tabs = hw_specs.get_activation_tables("gen3")
    if keep in tabs and need <= tabs[keep]:
        for name, s in tabs.items():
            if name != keep:
                s -= need

    nch = t // C              # chunks
    ngr = t // 512            # 512-token slices / chunk groups of 4
    npair = ngr // 2
    assert t % 1024 == 0      # gate chain packs slices in pairs

    nc = bacc.Bacc("TRN2", target_bir_lowering=False, debug=False)

    xt_d = nc.dram_tensor("xt", [128, 4, t], BF16, kind="ExternalInput")
    wqk_d = nc.dram_tensor("wqk", [128, 4, 2 * dk], BF16, kind="ExternalInput")
    wv_d = nc.dram_tensor("wv", [128, 4, dv], BF16, kind="ExternalInput")
    wg_d = nc.dram_tensor("wg", [128, 4, dv], BF16, kind="ExternalInput")
    wgk_d = nc.dram_tensor("wgk12", [128, 4, dk], BF16, kind="ExternalInput")
    wf_d = nc.dram_tensor("wfused", [dv, 10], BF16, kind="ExternalInput")
    nb_d = nc.dram_tensor("nbgk2", [128, 1], F32, kind="ExternalInput")
    um_d = nc.dram_tensor("umask", [C, C], F32, kind="ExternalInput")
    id_d = nc.dram_tensor("identb", [128, 128], BF16, kind="ExternalInput")
    idf_d = nc.dram_tensor("identf4", [4, 4], F32, kind="ExternalInput")
    out_d = nc.dram_tensor("out10", [t, 10], F32, kind="ExternalOutput")

    with tile.TileContext(nc) as tc:
        with (
            tc.tile_pool(name="wt", bufs=1) as wt,
            tc.tile_pool(name="big", bufs=1) as big,
            tc.tile_pool(name="sm", bufs=5) as sm,
            tc.tile_pool(name="e2", bufs=4) as e2,
            tc.tile_pool(name="pp", bufs=2, space="PSUM") as pp,
            tc.tile_pool(name="pc", bufs=4, space="PSUM") as pc,
            tc.tile_pool(name="ps", bufs=1, space="PSUM") as ps,
            tc.tile_pool(name="pq", bufs=1, space="PSUM") as pq,
        ):
            # ---- weights / consts; gate weights + x on the sync queue,
            # the rest spread over other engine queues so nothing blocks
            # the first gate matmul ----
            wqk_sb = wt.tile([128, 4, 2 * dk], BF16)
            wv_sb = wt.tile([128, 4, dv], BF16)
            wg_sb = wt.tile([128, 4, dv], BF16)
            wgk_sb = wt.tile([128, 4, dk], BF16)
            wf_sb = wt.tile([dv, 10], BF16)
            nb_sb = wt.tile([128, 1], F32)
            um_sb = wt.tile([C, C], F32)
            idb_sb = wt.tile([128, 128], BF16)
            idf_sb = wt.tile([4, 4], F32)
            nc.gpsimd.dma_start(wgk_sb[:], wgk_d[:])
            nc.gpsimd.dma_start(nb_sb[:], nb_d[:])
            nc.gpsimd.dma_start(wqk_sb[:], wqk_d[:])
            nc.gpsimd.dma_start(wv_sb[:], wv_d[:])
            nc.gpsimd.dma_start(wg_sb[:], wg_d[:])
            nc.gpsimd.dma_start(um_sb[:], um_d[:])
            nc.gpsimd.dma_start(idb_sb[:], id_d[:])
            nc.gpsimd.dma_start(idf_sb[:], idf_d[:])
            nc.gpsimd.dma_start(wf_sb[:], wf_d[:])
            eps_sb = wt.tile([128, 1], F32)
            nc.vector.memset(eps_sb[:], EPS)
            ones_sb = wt.tile([128, 1], F32)
            nc.vector.memset(ones_sb[:], 1.0)
            onesb_sb = wt.tile([128, 1], BF16)
            nc.vector.memset(onesb_sb[:], 1.0)

            # PE clock warmup: the HAM clock gate needs ~3.4us of sustained
            # PE activity to lift the 1.2->2.4 GHz throttle.  Burn the
            # startup DMA-wait window (~6.5..10.5us) on dependency-free junk
            # matmuls so the first real matmuls run at full clock.
            jnk_sb = wt.tile([128, 128], BF16)
            nc.vector.memset(jnk_sb[:], 0.0)
            pjnk = pq.tile([128, 128], F32, tag="p10")
            for _ in range(24):
                nc.tensor.matmul(pjnk[:], jnk_sb[:], jnk_sb[:],
                                 start=True, stop=True)

            # scan reset mask for the intra-chunk gate cumsum
            mres = wt.tile([128, 512], F32)
            nc.vector.memset(mres[:], 1.0)
            mres_v = mres[:].rearrange("p (c l) -> p c l", l=C)
            nc.vector.memset(mres_v[:, :, 0:1], 0.0)

            # ---- big SBUF tensors ----
            xT = big.tile([128, 4, t], BF16)      # x^T per 128-d-chunk
            qt = big.tile([dk, t], BF16)          # q-tilde transposed
            kt = big.tile([dk, t], BF16)          # k-tilde transposed
            spc = big.tile([128, t // 2], F32)    # packed gate cumsum
            dlast = big.tile([dk, nch], F32)
            vnat = big.tile([128, nch, dv], BF16)  # v natural per chunk
            sw = big.tile([dv, t], BF16)          # swish(g)^T
            atm = big.tile([C, nch, C], BF16)     # masked AT per chunk
            ktn = big.tile([C, nch, dk], BF16)    # k-tilde-scaled natural
            Sb = big.tile([dk, nch, dv], BF16)    # post-chunk states

            spc_v = spc[:].rearrange("p (c l) -> p c l", l=C)

            # ---- x^T load, split by d-chunk across both HWDGE queues so the
            # first gate matmul (which consumes d-chunks one at a time) can
            # start as soon as possible; weights interleave on the scalar
            # queue after slice 0.
            s0 = slice(0, 512)
            nc.sync.dma_start(xT[:, 0, s0], xt_d[:, 0, s0])
            nc.scalar.dma_start(xT[:, 2, s0], xt_d[:, 2, s0])
            nc.sync.dma_start(xT[:, 1, s0], xt_d[:, 1, s0])
            nc.scalar.dma_start(xT[:, 3, s0], xt_d[:, 3, s0])
            for j in range(1, ngr):
                ts = slice(j * 512, (j + 1) * 512)
                nc.sync.dma_start(xT[:, 0:2, ts], xt_d[:, 0:2, ts])
                nc.sync.dma_start(xT[:, 2:4, ts], xt_d[:, 2:4, ts])

            def emit_gate_pair(p):
                # two 512-token slices (2*p, 2*p+1) share the 128
                # partitions: rows 0:64 = even slice, 64:128 = odd
                ps0 = slice(2 * p * 512, (2 * p + 1) * 512)
                ps1 = slice((2 * p + 1) * 512, (2 * p + 2) * 512)
                pss = slice(p * 512, (p + 1) * 512)
                pgP = pp.tile([128, 512], F32, tag="P")
                for d4 in range(4):
                    nc.tensor.matmul(pgP[0:dk, :], wgk_sb[:, d4, :],
                                     xT[:, d4, ps0],
                                     start=(d4 == 0), stop=(d4 == 3))
                for d4 in range(4):
                    nc.tensor.matmul(pgP[dk:128, :], wgk_sb[:, d4, :],
                                     xT[:, d4, ps1],
                                     start=(d4 == 0), stop=(d4 == 3))
                eg = sm.tile([128, 512], F32, tag="eg")
                nc.scalar.activation(out=eg[:], in_=pgP[:], func=AF.Exp,
                                     scale=-1.0, bias=nb_sb[:])
                nc.scalar.activation(out=eg[:], in_=eg[:], func=AF.Ln,
                                     bias=ones_sb[:])
                nc.vector.tensor_tensor_scan(
                    out=spc[:, pss], data0=mres[:], data1=eg[:],
                    initial=0.0, op0=OP.mult, op1=OP.add)
                # per-chunk last-position decay, canonical [dk, nch] layout
                nc.scalar.activation(
                    out=dlast[:, 8 * p:8 * p + 4],
                    in_=spc_v[0:dk, 4 * p:4 * p + 4, C - 1:C],
                    func=AF.Exp, scale=-1.0 / GATE_NORM)
                nc.scalar.activation(
                    out=dlast[:, 8 * p + 4:8 * p + 8],
                    in_=spc_v[dk:128, 4 * p:4 * p + 4, C - 1:C],
                    func=AF.Exp, scale=-1.0 / GATE_NORM)

            def emit_slice(j):
                ts = slice(j * 512, (j + 1) * 512)
                half = slice(0, dk) if j % 2 == 0 else slice(dk, 128)
                pss = slice((j // 2) * 512, (j // 2 + 1) * 512)
                spc_j = spc[:, pss][half, :]

                # decay factors, stacked [q-rows | k-rows] to match pqk
                ee = sm.tile([128, 512], F32, tag="ee")
                nc.scalar.activation(out=ee[0:dk, :], in_=spc_j,
                                     func=AF.Exp, scale=-1.0 / GATE_NORM)
                nc.scalar.activation(out=ee[dk:128, :], in_=spc_j,
                                     func=AF.Exp, scale=1.0 / GATE_NORM)

                # q|k projection, decay applied on psum eviction -> bf16
                pqk = pp.tile([128, 512], F32, tag="P")
                for d4 in range(4):
                    nc.tensor.matmul(pqk[:], wqk_sb[:, d4, :], xT[:, d4, ts],
                                     start=(d4 == 0), stop=(d4 == 3))
                nc.vector.tensor_mul(out=qt[:, ts], in0=pqk[0:dk, :],
                                     in1=ee[0:dk, :])
                nc.vector.tensor_mul(out=kt[:, ts], in0=pqk[dk:128, :],
                                     in1=ee[dk:128, :])

                # g^T projection + swish (g never leaves PSUM)
                pgt = pp.tile([dv, 512], F32, tag="P")
                for d4 in range(4):
                    nc.tensor.matmul(pgt[:], wg_sb[:, d4, :], xT[:, d4, ts],
                                     start=(d4 == 0), stop=(d4 == 3))
                s1 = sm.tile([dv, 512], F32, tag="s1")
                nc.scalar.activation(out=s1[:], in_=pgt[:], func=AF.Exp,
                                     scale=-1.0)
                nc.vector.tensor_scalar_add(out=s1[:], in0=s1[:], scalar1=1.0)
                nc.vector.reciprocal_approx_fast(out=s1[:], in_=s1[:])
                nc.vector.tensor_mul(out=sw[:, ts], in0=s1[:], in1=pgt[:])

                # v projection: transposed matmul (4 wide MMs instead of 16
                # narrow ones), then PE transposes back to natural layout
                pvT = pp.tile([dv, 512], F32, tag="P")
                for d4 in range(4):
                    nc.tensor.matmul(pvT[:], wv_sb[:, d4, :], xT[:, d4, ts],
                                     start=(d4 == 0), stop=(d4 == 3))
                vTs = sm.tile([dv, 512], BF16, tag="vTs")
                nc.scalar.copy(vTs[:], pvT[:])
                pvn = pc.tile([128, 4, dv], BF16, tag="C")
                for i in range(4):
                    nc.tensor.transpose(pvn[:, i, :], vTs[:, i * C:(i + 1) * C],
                                        idb_sb[:])
                nc.scalar.copy(vnat[:, 4 * j:4 * j + 4, :], pvn[:])

                # intra-chunk attention AT = kt_c^T-block qt_c-block, masked
                pat4 = pc.tile([C, 4, C], F32, tag="C")
                for i in range(4):
                    cs = slice((4 * j + i) * C, (4 * j + i + 1) * C)
                    nc.tensor.matmul(pat4[:, i, :], kt[:, cs], qt[:, cs],
                                     start=True, stop=True)
                nc.vector.tensor_mul(
                    out=atm[:, 4 * j:4 * j + 4, :], in0=pat4[:],
                    in1=um_sb[:][:, None, :].broadcast_to([C, 4, C]))

                # k-tilde scaled by e_c, transposed to natural -> ktn
                kts = sm.tile([dk, 512], BF16, tag="kts")
                kts_v = kts[:].rearrange("p (c l) -> p c l", l=C)
                nc.vector.tensor_mul(
                    out=kts_v[:],
                    in0=kt[:, ts].rearrange("p (c l) -> p c l", l=C),
                    in1=dlast[:][:, 4 * j:4 * j + 4, None]
                        .broadcast_to([dk, 4, C]))
                pkt4 = pc.tile([C, 4, dk], BF16, tag="C")
                for i in range(4):
                    nc.tensor.transpose(pkt4[:, i, :], kts_v[:, i, :],
                                        idb_sb[0:dk, 0:dk])
                nc.scalar.copy(ktn[:, 4 * j:4 * j + 4, :], pkt4[:])

                # D'_c = ktn_c^T @ V_c (e_c already folded into ktn);
                # state chain S_c = S_{c-1}*e_c + D'_c straight from PSUM
                pds4 = pc.tile([dk, 4, dv], F32, tag="C")
                for i in range(4):
                    tt = 4 * j + i
                    nc.tensor.matmul(pds4[:, i, :], ktn[:, tt, :],
                                     vnat[:, tt, :], start=True, stop=True)
                for i in range(4):
                    tt = 4 * j + i
                    if tt == 0:
                        nc.vector.tensor_copy(out=Sb[:, 0, :],
                                              in_=pds4[:, 0, :])
                    else:
                        nc.vector.scalar_tensor_tensor(
                            out=Sb[:, tt, :], in0=Sb[:, tt - 1, :],
                            scalar=dlast[:, tt:tt + 1], in1=pds4[:, i, :],
                            op0=OP.mult, op1=OP.add)

            # ---- phase 2 tiles (persist across groups) ----
            # indicator columns: ind[:, g, m] = (m == g), for the rmsnorm
            # sum-of-squares matmuls (ones-column selects output row g)
            ind_sb = wt.tile([128, 4, 4], BF16)
            nc.vector.memset(ind_sb[:], 0.0)
            for g in range(ngr):
                nc.vector.memset(ind_sb[:, g, g:g + 1], 1.0)
            pms = ps.tile([4, 512], F32, tag="ms")
            p10 = pq.tile([C, nch, 10], F32, tag="p10")

            def emit_group(g):
                gs = slice(g * 512, (g + 1) * 512)
                po4 = pc.tile([dv, 4, C], F32, tag="C")
                for i in range(4):
                    cc = 4 * g + i
                    cs = slice(cc * C, (cc + 1) * C)
                    first = (cc == 0)
                    nc.tensor.matmul(po4[:, i, :], vnat[:, cc, :],
                                     atm[:, cc, :], start=True, stop=first)
                    if not first:
                        nc.tensor.matmul(po4[:, i, :], Sb[:, cc - 1, :],
                                         qt[:, cs], start=False, stop=True)
                sq4 = e2.tile([dv, 512], BF16, tag="sq4")
                nc.scalar.activation(out=sq4[:], in_=po4[:], func=AF.Square)
                ot4 = e2.tile([dv, 512], BF16, tag="ot4")
                nc.vector.tensor_mul(out=ot4[:], in0=po4[:], in1=sw[:, gs])
                # row g of pms accumulates this group's per-chunk sum-sq
                nc.tensor.matmul(pms[:], ind_sb[:, g, :], sq4[:],
                                 start=(g == 0), stop=(g == ngr - 1))
                for i in range(4):
                    nc.tensor.matmul(p10[:, 4 * g + i, :],
                                     ot4[:, i * C:(i + 1) * C],
                                     wf_sb[:], start=True, stop=True)

            # ---- emission: gate pairs ahead; group g right after slice g
            # (it depends only on that slice's projections + states) ----
            emit_gate_pair(0)
            emit_slice(0)
            emit_group(0)
            emit_slice(1)
            emit_group(1)
            for p in range(1, npair):
                emit_gate_pair(p)
                emit_slice(2 * p)
                emit_group(2 * p)
                emit_slice(2 * p + 1)
                emit_group(2 * p + 1)

            # rmsnorm rstd for all chunks at once (in [group, (chunk, t)]
            # layout), PE-transposed to per-token layout, one gated eviction
            lnv = e2.tile([4, 512], F32, tag="lnv")
            nc.scalar.activation(out=lnv[:], in_=pms[:], func=AF.Ln,
                                 scale=1.0 / dv, bias=eps_sb[0:4])
            rstd = e2.tile([4, 512], F32, tag="rstd")
            nc.scalar.activation(out=rstd[:], in_=lnv[:], func=AF.Exp,
                                 scale=-0.5)
            prs = ps.tile([C, 4, 4], F32, tag="ms")
            for i in range(4):
                nc.tensor.transpose(prs[:, i, :], rstd[:, i * C:(i + 1) * C],
                                    idf_sb[:])
            rstd_t = e2.tile([C, 4, 4], F32, tag="rstd_t")
            nc.scalar.copy(rstd_t[:], prs[:])
            o10 = e2.tile([C, nch, 10], F32, tag="o10")
            nc.vector.tensor_mul(
                out=o10[:].rearrange("p (g i) j -> p g i j", g=ngr, i=4),
                in0=p10[:].rearrange("p (g i) j -> p g i j", g=ngr, i=4),
                in1=rstd_t[:].transpose([0, 2, 1])[:, 0:ngr, :, None]
                    .broadcast_to([C, ngr, 4, 10]))
            nc.sync.dma_start(
                out_d[:].rearrange("(c p) j -> p c j", c=nch), o10[:])

    nc.compile()
    return nc


def _prep_inputs(inputs, t=T):
    """Per-core input dicts: core = 4*b + h."""
    from ml_dtypes import bfloat16
    ins = {k: np.ascontiguousarray(np.asarray(v, dtype=np.float32))
           for k, v in inputs.items()}
    x, Wq, Wk, Wv, Wg = ins["x"], ins["Wq"], ins["Wk"], ins["Wv"], ins["Wg"]
    Wgk12 = (ins["Wgk1"].astype(np.float64) @ ins["Wgk2"].astype(np.float64))
    bgk2, gnorm = ins["bgk2"], ins["gnorm_w"]
    Wo, Whead = ins["Wo"], ins["Whead"]

    um = (np.arange(C)[:, None] <= np.arange(C)[None, :]).astype(np.float32)
    identb = np.eye(128, dtype=bfloat16)
    identf4 = np.eye(4, dtype=np.float32)

    def chunk_w(w):  # [512, n] -> [128, 4, n] bf16
        return np.ascontiguousarray(
            w.reshape(4, 128, -1).transpose(1, 0, 2)).astype(bfloat16)

    in_maps = []
    for core in range(8):
        b, h = divmod(core, 4)
        wf = ((gnorm[:, None].astype(np.float64)
               * Wo[h * dv:(h + 1) * dv, :].astype(np.float64))
              @ Whead.astype(np.float64)).astype(np.float32)
        nb = -bgk2[h * dk:(h + 1) * dk, None]
        in_maps.append({
            "xt": np.ascontiguousarray(
                x[b, :t].T.reshape(4, 128, t).transpose(1, 0, 2)
            ).astype(bfloat16),
            "wqk": chunk_w(np.concatenate(
                [Wq[:, h * dk:(h + 1) * dk] * SCALE,
                 Wk[:, h * dk:(h + 1) * dk]], 1)),
            "wv": chunk_w(Wv[:, h * dv:(h + 1) * dv]),
            "wg": chunk_w(Wg[:, h * dv:(h + 1) * dv]),
            "wgk12": chunk_w(Wgk12[:, h * dk:(h + 1) * dk].astype(np.float32)),
            "wfused": np.ascontiguousarray(wf).astype(bfloat16),
            "nbgk2": np.ascontiguousarray(np.concatenate([nb, nb], 0)),
            "umask": um,
            "identb": identb,
            "identf4": identf4,
        })
    return in_maps


def _gather(results, inputs, t=T):
    bhead = np.asarray(inputs["bhead"], dtype=np.float32)
    out = np.zeros((B, t, 10), np.float32)
    for core in range(8):
        b = core // 4
        out[b] += results[core]["out10"]
    out += bhead[None, None, :]
    return out


def run(inputs, trace=False, **kw):
    from concourse.bass_utils import run_bass_kernel_spmd
    if "nc" not in _CACHE:
        _CACHE["nc"] = build()
    nc = _CACHE["nc"]
    in_maps = _prep_inputs(inputs)
    res = run_bass_kernel_spmd(nc, in_maps, core_ids=list(range(8)),
                               trace=trace, **kw)
    return _gather(res.results, inputs), res


def kernel(**inputs) -> np.ndarray:
    out, _ = run(inputs, trace=False)
    return out
